# revision 35
# baseline (speedup 1.0000x reference)
"""CVRP decoder kernel for Trainium2 (8 NeuronCores, batch-data-parallel).

Computes, per batch b (B=64, P=64, N=1000, H=128):
    q_graph   = mean_n(emb) @ Wq_graph
    q_first   = encoded_q1 @ Wq_first
    q_last    = emb[last_node] @ Wq_last
    q_visited = (vis01 @ emb / N) @ W_visited          (vis01 = isneginf(mask))
    final_q   = sum of the above + load*W_load + b_load
    score     = final_q @ emb^T / sqrt(H) - dists[last_node] / sqrt(2)
    probs     = softmax(10*tanh(score) + (-BIG if visited))

Sharding: batch across the 8 cores (pure data parallel); the 8 batches per
core run as 4 pairs of 2 batches stacked on the 128 SBUF partitions.

Device kernel design (per core):
  * All matmul operands 16/8-bit, fp32 PSUM: emb n-major + visited mask
    in fp8e4m3 (qv/mean matmuls), emb h-major + final_q in fp16 (score).
  * The dist bias and the additive visited mask are folded into the score
    PSUM with identity-lhsT matmuls, so tanh reads PSUM directly and the
    whole mask/bias elementwise stage disappears.  The mask rides INSIDE
    the tanh argument (-96 saturates tanh to -1; exp(-10)=4.5e-5 is far
    below the accuracy gate).
  * dists rows are gathered on-device by last_node (indirect DMA, rows
    padded >2048B so they ride the software DGE ring which overlaps the
    bulk loads; the hardware-dynamic indirect ring is starved until all
    direct loads drain).
  * Loads are split across the two HWDGE dispatch rings (sync + scalar
    engines) plus the gpsimd software ring, all issued up front with no
    per-pair store-waits blocking later loads.
  * Stages are software-pipelined across pairs (A=qv/mean/lastemb^T,
    B=final_q, C=score+softmax+store) so each engine's small ops are
    queued ahead of the next pair's long activations.

Host-side prep inside kernel() (plain numpy, layout/dtype only): fp16/fp8
casts, transposed layouts (emb^T, mask^T, eq1^T), flat gather indices,
constant folding (-dists/sqrt2, b_load/sqrt(H)); all matmuls, gathers and
the softmax run on device.
"""

import json
import math
import numpy as np
import ml_dtypes
from contextlib import ExitStack

import concourse.bass as bass
import concourse.mybir as mybir
import concourse.tile as tile
from concourse.bass_utils import run_bass_kernel_spmd
from concourse.masks import make_identity


def _split_excess_waits(bir_bytes: bytes, max_waits: int = 1) -> bytes:
    """Walrus in this image rejects instructions carrying too many sem waits
    ("Too many sync wait commands", e.g. on Tile's kernel-tail Drain).
    Hoist excess waits onto preceding same-engine EventSemaphore carriers
    (pure sync ops) — sems are monotonic, so a chain of instructions whose
    waits partition the original list is equivalent."""
    d = json.loads(bir_bytes)
    n = [0]
    for fn in d.get("functions", []):
        for blk in fn.get("blocks", []):
            out = []
            for ins in blk.get("instructions", []):
                si = ins.get("sync_info") or {}
                waits = si.get("on_wait") or []
                if len(waits) > max_waits:
                    extra, keep = waits[:-max_waits], waits[-max_waits:]
                    ins["sync_info"]["on_wait"] = keep
                    for i in range(0, len(extra), max_waits):
                        n[0] += 1
                        carrier = {
                            "name": f"I-waitsplit-{n[0]}",
                            "opcode": "EventSemaphore",
                            "engine": ins["engine"],
                            "ins": [],
                            "outs": [],
                            "sync_info": {
                                "on_update": [],
                                "on_wait": extra[i:i + max_waits],
                            },
                        }
                        if "debug" in ins:
                            carrier["debug"] = ins["debug"]
                        out.append(carrier)
                out.append(ins)
            blk["instructions"] = out
    return json.dumps(d).encode()


def _install_walrus_shim():
    import concourse.bass2jax as b2j
    import concourse.bass_utils as bu
    if getattr(bu, "_waitsplit_installed", False):
        return
    real = bu.compile_bir_kernel

    def patched(bir_json, tmpdir, neff_name="file.neff", **kw):
        if isinstance(bir_json, (bytes, bytearray, str)):
            if isinstance(bir_json, str):
                bir_json = bir_json.encode()
            bir_json = _split_excess_waits(bir_json)
        return real(bir_json, tmpdir, neff_name=neff_name, **kw)

    bu.compile_bir_kernel = patched
    b2j.compile_bir_kernel = patched
    bu._waitsplit_installed = True


_install_walrus_shim()

F32 = mybir.dt.float32
F8 = mybir.dt.float8e4
F16 = mybir.dt.float16
I32 = mybir.dt.int32
OP = mybir.AluOpType
AF = mybir.ActivationFunctionType

B, P, N, H = 64, 64, 1000, 128
NCORES = 8
NB = B // NCORES          # 8 batches per core
NPAIR = NB // 2           # 4 pairs
NCHUNK = 8                # 8 n-chunks of 128 (n padded 1000 -> 1024)
NPAD = 1024
NDPAD = 1088              # dists rows padded >2048B so the gather rides SWDGE

MASK_NEG = -96.0          # visited bias inside tanh arg (saturates tanh to -1);
                          # exactly representable in fp8e4m3
INV_N = 1.0 / N
FQ2 = 1.0 / math.sqrt(H)
INV_SQRT2 = 1.0 / math.sqrt(2.0)
TANH_CLIP = 10.0


def build_nc():
    nc = bass.Bass()

    # fp16 inputs, host-prepared layouts (see _shard_inputs)
    embN = nc.dram_tensor("embN", [128, NB * NCHUNK * H], F8,
                          kind="ExternalInput")     # (p,(b,c,h)) n=128c+p
    embT = nc.dram_tensor("embT", [128, NB * N], F16,
                          kind="ExternalInput")     # (h,(b,n))
    embF = nc.dram_tensor("embF", [NB * N, H], F16,
                          kind="ExternalInput")     # flat n-major (gather)
    distsF = nc.dram_tensor("distsF", [NB * N, NDPAD], F16,
                            kind="ExternalInput")   # pre-scaled by 1/sqrt(2)
    maskP = nc.dram_tensor("maskP", [128, NPAIR * N], F8,
                           kind="ExternalInput")    # (p2,(pr,n)) {0,-96}
    maskT = nc.dram_tensor("maskT", [128, NB * NCHUNK * 65], F8,
                           kind="ExternalInput")    # (p,(b,c,q)) q<64: vis01
    idxt = nc.dram_tensor("idxt", [128, NPAIR], I32,
                          kind="ExternalInput")     # flat row idx +1000*b
    cst16 = nc.dram_tensor("cst16", [128, 1664], F16,
                           kind="ExternalInput")    # wcat|eq1T|loadv(row0)
    bldT = nc.dram_tensor("bldT", [H, 1], F32,
                          kind="ExternalInput")     # b_load / sqrt(H)
    probs = nc.dram_tensor("probs", [128, NPAIR * N], F16,
                           kind="ExternalOutput")   # (p2,(pr,n))

    with tile.TileContext(nc) as tc:
        with ExitStack() as ctx:
            const = ctx.enter_context(tc.tile_pool(name="const", bufs=1))
            sb = ctx.enter_context(tc.tile_pool(name="sb", bufs=1))
            ps_big = ctx.enter_context(
                tc.tile_pool(name="ps_big", bufs=4, space="PSUM"))
            ps_small = ctx.enter_context(
                tc.tile_pool(name="ps_small", bufs=2, space="PSUM"))

            # ---- constants ----
            ident = const.tile([128, 128], F16, tag="ident")
            make_identity(nc, ident[:])
            ident8 = const.tile([128, 128], F8, tag="ident8")
            make_identity(nc, ident8[:])

            idx_s = const.tile([128, NPAIR], I32, tag="idx_s")
            nc.sync.dma_start(idx_s[:], idxt[:])
            cst = const.tile([128, 1664], F16, tag="cst")
            nc.sync.dma_start(cst[:], cst16[:])
            bld_s = const.tile([H, 1], F32, tag="bld_s")
            nc.sync.dma_start(bld_s[:], bldT[:])
            wcat_s = cst[:, 0:640]

            # ---- all gathers up front (gpsimd dispatch; depend on idx).
            # The indirect ring serves rows serially in dispatch order, so
            # interleave per pair: each pair's rows land before the next's.
            lastemb = sb.tile([128, NPAIR, H], F16, tag="lastemb")
            for pr in range(NPAIR):
                nc.gpsimd.indirect_dma_start(
                    out=lastemb[:, pr, :], out_offset=None, in_=embF[:],
                    in_offset=bass.IndirectOffsetOnAxis(
                        ap=idx_s[:, pr:pr + 1], axis=0))
            distg = []
            for pr in range(NPAIR):
                dg = sb.tile([128, NDPAD], F16, tag=f"distg{pr}")
                nc.gpsimd.indirect_dma_start(
                    out=dg[:], out_offset=None, in_=distsF[:],
                    in_offset=bass.IndirectOffsetOnAxis(
                        ap=idx_s[:, pr:pr + 1], axis=0))
                distg.append(dg)

            # ---- all big loads up front (sync dispatch, no waits) ----
            embNs, maskTs, embTs, mps = [], [], [], []
            for pr in range(NPAIR):
                en = sb.tile([128, 2, NCHUNK, H], F8, tag=f"embN{pr}")
                nc.sync.dma_start(en[:], embN[
                    :, pr * 2 * NCHUNK * H:(pr + 1) * 2 * NCHUNK * H]
                    .rearrange("p (b c h) -> p b c h", b=2, c=NCHUNK))
                embNs.append(en)
                mt = sb.tile([128, 2, NCHUNK, 65], F8, tag=f"maskT{pr}")
                nc.sync.dma_start(mt[:], maskT[
                    :, pr * 2 * NCHUNK * 65:(pr + 1) * 2 * NCHUNK * 65]
                    .rearrange("p (b c q) -> p b c q", b=2, c=NCHUNK))
                maskTs.append(mt)
                et = sb.tile([128, 2, N], F16, tag=f"embT{pr}")
                eng = nc.sync if pr < 2 else nc.scalar
                eng.dma_start(et[:], embT[
                    :, pr * 2 * N:(pr + 1) * 2 * N]
                    .rearrange("h (b n) -> h b n", b=2))
                embTs.append(et)
                mp = sb.tile([128, N], F8, tag=f"maskP{pr}")
                nc.scalar.dma_start(mp[:], maskP[:, pr * N:(pr + 1) * N])
                mps.append(mp)

            # ---- per-pair compute: staged software pipeline ----
            # A: qv matmuls + evicts, lastemb^T, emb^T (PE transposes)
            # B: final_q matmuls + fqT evict
            # C: score + softmax + store
            qvm_t, meanrep_t, lastembT_t, fqT_t = {}, {}, {}, {}

            def stage_A(pr):
                # q_visited pre + mean(emb): psum [h, 64+1] per batch
                qvm = sb.tile([128, 2, 64], F16, tag=f"qvm{pr}", name="qvm")
                meanrep = sb.tile([128, 128], F16, tag=f"meanrep{pr}",
                                  name="meanrep")
                for j in range(2):
                    pqv = ps_small.tile([128, 65], F32, tag="pqv", name="pqv")
                    for c in range(NCHUNK):
                        nc.tensor.matmul(
                            pqv[:],
                            lhsT=embNs[pr][:, j, c, :],
                            rhs=maskTs[pr][:, j, c, :],
                            start=(c == 0), stop=(c == NCHUNK - 1))
                    nc.vector.tensor_scalar(
                        out=qvm[:, j, :], in0=pqv[:, 0:64],
                        scalar1=INV_N, scalar2=None, op0=OP.mult)
                    nc.vector.tensor_scalar(
                        out=meanrep[:, 64 * j:64 * j + 64],
                        in0=pqv[:, 64:65].to_broadcast([128, 64]),
                        scalar1=INV_N, scalar2=None, op0=OP.mult)
                qvm_t[pr], meanrep_t[pr] = qvm, meanrep

                # lastemb^T via PE
                psl = ps_small.tile([128, 128], F16, tag="psL", bufs=1,
                                    name="psl")
                nc.tensor.transpose(out=psl[:], in_=lastemb[:, pr, :],
                                    identity=ident[:])
                lastembT = sb.tile([128, 128], F16, tag=f"lastembT{pr}",
                                   name="lastembT")
                nc.vector.tensor_copy(out=lastembT[:], in_=psl[:])
                lastembT_t[pr] = lastembT

            def stage_B(pr):
                pfq = ps_small.tile([128, 128], F32, tag="pfq", bufs=1,
                                    name="pfq")
                nc.tensor.matmul(pfq[:], lhsT=wcat_s[:, 0:128],
                                 rhs=cst[:, 640 + 128 * pr:768 + 128 * pr],
                                 start=True, stop=False)
                nc.tensor.matmul(pfq[:], lhsT=wcat_s[:, 128:256],
                                 rhs=lastembT_t[pr][:], start=False,
                                 stop=False)
                nc.tensor.matmul(pfq[:], lhsT=wcat_s[:, 256:384],
                                 rhs=meanrep_t[pr][:], start=False, stop=False)
                nc.tensor.matmul(pfq[:], lhsT=wcat_s[:, 384:512],
                                 rhs=qvm_t[pr][:], start=False, stop=False)
                nc.tensor.matmul(pfq[:], lhsT=wcat_s[0:1, 512:640],
                                 rhs=cst[0:1, 1152 + 128 * pr:
                                         1280 + 128 * pr],
                                 start=False, stop=True)
                # fqT = psum/sqrt(H) + b_load/sqrt(H)
                fqT = sb.tile([128, 128], F16, tag=f"fqT{pr}", name="fqT")
                nc.vector.scalar_tensor_tensor(
                    out=fqT[:], in0=pfq[:], scalar=FQ2,
                    in1=bld_s[:, 0:1].to_broadcast([128, 128]),
                    op0=OP.mult, op1=OP.add)
                fqT_t[pr] = fqT

            pout = sb.tile([128, NPAIR, N], F16, tag="pout")

            def stage_C(pr):
                # score psum also absorbs -dist/sqrt2 (lhsT=-I) and the
                # {0,-1000} visited bias (lhsT=I): tanh saturates to -1 and
                # exp(-10) ~ 4.5e-5, well under the accuracy gate, so the
                # mask can ride inside the tanh argument.
                t = sb.tile([128, N], F16, tag=f"t{pr}", name="t")
                for (n0, n1) in ((0, 512), (512, N)):
                    psc = ps_big.tile([128, n1 - n0], F32, tag="psc",
                                      name="psc")
                    for j in range(2):
                        nc.tensor.matmul(
                            psc[64 * j:64 * j + 64, :],
                            lhsT=fqT_t[pr][:, 64 * j:64 * j + 64],
                            rhs=embTs[pr][:, j, n0:n1],
                            start=True, stop=False, skip_group_check=True)
                    nc.tensor.matmul(
                        psc[:], lhsT=ident[:], rhs=distg[pr][:, n0:n1],
                        start=False, stop=False, skip_group_check=True)
                    nc.tensor.matmul(
                        psc[:], lhsT=ident8[:], rhs=mps[pr][:, n0:n1],
                        start=False, stop=True, skip_group_check=True)
                    nc.scalar.activation(t[:, n0:n1], psc[:], AF.Tanh)

                e = sb.tile([128, N], F16, tag=f"e{pr}", name="e")
                ssum = sb.tile([128, 1], F32, tag=f"ssum{pr}", name="ssum")
                nc.scalar.activation(e[:], t[:], AF.Exp, scale=TANH_CLIP,
                                     accum_out=ssum[:])
                rec = sb.tile([128, 1], F32, tag=f"rec{pr}", name="rec")
                nc.vector.reciprocal(out=rec[:], in_=ssum[:])
                nc.vector.tensor_scalar(out=pout[:, pr, :], in0=e[:],
                                        scalar1=rec[:, 0:1], scalar2=None,
                                        op0=OP.mult)
                if pr % 2 == 1:
                    nc.sync.dma_start(
                        probs[:, (pr - 1) * N:(pr + 1) * N],
                        pout[:, pr - 1:pr + 1, :])

            stage_A(0)
            for pr in range(NPAIR):
                stage_B(pr)
                if pr + 1 < NPAIR:
                    stage_A(pr + 1)
                stage_C(pr)

    return nc


_CACHE = {}


def _get_nc():
    if "nc" not in _CACHE:
        _CACHE["nc"] = build_nc()
    return _CACHE["nc"]


def _shard_inputs(inputs):
    f16 = np.float16
    f8 = ml_dtypes.float8_e4m3
    dists = np.asarray(inputs["dists"], dtype=np.float32)
    embeddings = np.asarray(inputs["embeddings"], dtype=np.float32)
    encoded_q1 = np.asarray(inputs["encoded_q1"], dtype=np.float32)
    last_node = np.asarray(inputs["last_node"]).astype(np.int64)
    load = np.asarray(inputs["load"], dtype=np.float32)
    mask = np.asarray(inputs["group_ninf_mask"], dtype=np.float32)
    vis_all = (np.isneginf(mask) | (mask < -1e30))

    wcat = np.zeros((H, 640), f16)
    wcat[:, 0:128] = inputs["Wq_first"].astype(f16)
    wcat[:, 128:256] = inputs["Wq_last"].astype(f16)
    wcat[:, 256:384] = inputs["Wq_graph"].astype(f16)
    wcat[:, 384:512] = inputs["W_visited"].astype(f16)
    wcat[0, 512:640] = inputs["W_load"].astype(f16)
    bldT = (np.asarray(inputs["b_load"], dtype=np.float32) * FQ2) \
        .astype(np.float32).reshape(H, 1)

    in_maps = []
    for c in range(NCORES):
        s = slice(c * NB, (c + 1) * NB)
        emb = embeddings[s]                          # [8,1000,128]
        embT = np.ascontiguousarray(
            emb.transpose(2, 0, 1)).astype(f16).reshape(128, NB * N)
        embp = np.zeros((NB, NPAD, H), f8)
        embp[:, :N] = emb.astype(f8)
        embN = np.ascontiguousarray(
            embp.reshape(NB, NCHUNK, 128, H).transpose(2, 0, 1, 3)
        ).reshape(128, NB * NCHUNK * H)
        embF = np.ascontiguousarray(emb.reshape(NB * N, H).astype(f16))
        distsF = np.zeros((NB * N, NDPAD), f16)
        distsF[:, :N] = (dists[s].reshape(NB * N, N)
                         * np.float32(-INV_SQRT2)).astype(f16)

        vis = vis_all[s]                             # [8,64,1000] bool
        maskP = np.ascontiguousarray(
            (vis.reshape(NPAIR, 128, N).transpose(1, 0, 2))
            .astype(np.float32) * np.float32(MASK_NEG)
        ).astype(f8).reshape(128, NPAIR * N)
        visp = np.zeros((NB, NPAD, P), f8)
        visp[:, :N] = vis.transpose(0, 2, 1).astype(f8)
        maskT = np.concatenate(
            [visp.reshape(NB, NCHUNK, 128, P).transpose(2, 0, 1, 3),
             np.ones((128, NB, NCHUNK, 1), f8)],
            axis=3).reshape(128, NB * NCHUNK * 65)
        maskT = np.ascontiguousarray(maskT)

        eq1T = np.ascontiguousarray(
            encoded_q1[s].astype(f16).transpose(2, 0, 1)
        ).reshape(128, NPAIR * 128)
        idxt = np.ascontiguousarray(
            (last_node[s] + np.arange(NB)[:, None] * N)
            .astype(np.int32).reshape(NPAIR, 128).T)
        loadv = load[s].astype(f16).reshape(1, NPAIR * 128)
        cst16 = np.zeros((128, 1664), f16)
        cst16[:, 0:640] = wcat
        cst16[:, 640:1152] = eq1T
        cst16[0, 1152:1664] = loadv[0]

        in_maps.append(dict(
            embN=embN, embT=embT, embF=embF, distsF=distsF,
            maskP=maskP, maskT=maskT, idxt=idxt,
            cst16=cst16, bldT=bldT,
        ))
    return in_maps


def _run(inputs, trace=False, **kw):
    nc = _get_nc()
    in_maps = _shard_inputs(inputs)
    res = run_bass_kernel_spmd(nc, in_maps, list(range(NCORES)),
                               trace=trace, **kw)
    out = np.concatenate(
        [r["probs"].astype(np.float32).reshape(128, NPAIR, N)
         .transpose(1, 0, 2).reshape(NB, P, N)
         for r in res.results], axis=0)
    return out, res


def kernel(**inputs) -> np.ndarray:
    out, _ = _run(inputs)
    return out


# revision 36
# speedup vs baseline: 1.0463x; 1.0463x over previous
"""CVRP decoder kernel for Trainium2 (8 NeuronCores, batch-data-parallel).

Computes, per batch b (B=64, P=64, N=1000, H=128):
    q_graph   = mean_n(emb) @ Wq_graph
    q_first   = encoded_q1 @ Wq_first
    q_last    = emb[last_node] @ Wq_last
    q_visited = (vis01 @ emb / N) @ W_visited          (vis01 = isneginf(mask))
    final_q   = sum of the above + load*W_load + b_load
    score     = final_q @ emb^T / sqrt(H) - dists[last_node] / sqrt(2)
    probs     = softmax(10*tanh(score) + (-BIG if visited))

Sharding: batch across the 8 cores (pure data parallel); the 8 batches per
core run as 4 pairs of 2 batches stacked on the 128 SBUF partitions.

Device kernel design (per core):
  * All matmul operands 16/8-bit, fp32 PSUM: emb n-major + visited mask
    in fp8e4m3 (qv/mean matmuls), emb h-major + final_q in fp16 (score).
  * The dist bias and the additive visited mask are folded into the score
    PSUM with identity-lhsT matmuls, so tanh reads PSUM directly and the
    whole mask/bias elementwise stage disappears.  The mask rides INSIDE
    the tanh argument (-96 saturates tanh to -1; exp(-10)=4.5e-5 is far
    below the accuracy gate).
  * dists rows are gathered on-device by last_node (indirect DMA, rows
    padded >2048B so they ride the software DGE ring which overlaps the
    bulk loads; the hardware-dynamic indirect ring is starved until all
    direct loads drain).
  * Loads are split across the two HWDGE dispatch rings (sync + scalar
    engines) plus the gpsimd software ring, all issued up front with no
    per-pair store-waits blocking later loads.
  * Stages are software-pipelined across pairs (A=qv/mean/lastemb^T,
    B=final_q, C=score+softmax+store) so each engine's small ops are
    queued ahead of the next pair's long activations.

Host-side prep inside kernel() (plain numpy, layout/dtype only): fp16/fp8
casts, transposed layouts (emb^T, mask^T, eq1^T), flat gather indices,
constant folding (-dists/sqrt2, b_load/sqrt(H)); all matmuls, gathers and
the softmax run on device.
"""

import json
import math
import numpy as np
import ml_dtypes
from contextlib import ExitStack

import concourse.bass as bass
import concourse.mybir as mybir
import concourse.tile as tile
from concourse.bass_utils import run_bass_kernel_spmd
from concourse.masks import make_identity


def _split_excess_waits(bir_bytes: bytes, max_waits: int = 1) -> bytes:
    """Walrus in this image rejects instructions carrying too many sem waits
    ("Too many sync wait commands", e.g. on Tile's kernel-tail Drain).
    Hoist excess waits onto preceding same-engine EventSemaphore carriers
    (pure sync ops) — sems are monotonic, so a chain of instructions whose
    waits partition the original list is equivalent."""
    d = json.loads(bir_bytes)
    n = [0]
    for fn in d.get("functions", []):
        for blk in fn.get("blocks", []):
            out = []
            for ins in blk.get("instructions", []):
                si = ins.get("sync_info") or {}
                waits = si.get("on_wait") or []
                if len(waits) > max_waits:
                    extra, keep = waits[:-max_waits], waits[-max_waits:]
                    ins["sync_info"]["on_wait"] = keep
                    for i in range(0, len(extra), max_waits):
                        n[0] += 1
                        carrier = {
                            "name": f"I-waitsplit-{n[0]}",
                            "opcode": "EventSemaphore",
                            "engine": ins["engine"],
                            "ins": [],
                            "outs": [],
                            "sync_info": {
                                "on_update": [],
                                "on_wait": extra[i:i + max_waits],
                            },
                        }
                        if "debug" in ins:
                            carrier["debug"] = ins["debug"]
                        out.append(carrier)
                out.append(ins)
            blk["instructions"] = out
    return json.dumps(d).encode()


def _install_walrus_shim():
    import concourse.bass2jax as b2j
    import concourse.bass_utils as bu
    if getattr(bu, "_waitsplit_installed", False):
        return
    real = bu.compile_bir_kernel

    def patched(bir_json, tmpdir, neff_name="file.neff", **kw):
        if isinstance(bir_json, (bytes, bytearray, str)):
            if isinstance(bir_json, str):
                bir_json = bir_json.encode()
            bir_json = _split_excess_waits(bir_json)
        return real(bir_json, tmpdir, neff_name=neff_name, **kw)

    bu.compile_bir_kernel = patched
    b2j.compile_bir_kernel = patched
    bu._waitsplit_installed = True


_install_walrus_shim()

F32 = mybir.dt.float32
F8 = mybir.dt.float8e4
F16 = mybir.dt.float16
I32 = mybir.dt.int32
OP = mybir.AluOpType
AF = mybir.ActivationFunctionType

B, P, N, H = 64, 64, 1000, 128
NCORES = 8
NB = B // NCORES          # 8 batches per core
NPAIR = NB // 2           # 4 pairs
NCHUNK = 8                # 8 n-chunks of 128 (n padded 1000 -> 1024)
NPAD = 1024
NDPAD = 1088              # dists rows padded >2048B so the gather rides SWDGE

MASK_NEG = -96.0          # visited bias inside tanh arg (saturates tanh to -1);
                          # exactly representable in fp8e4m3
INV_N = 1.0 / N
FQ2 = 1.0 / math.sqrt(H)
INV_SQRT2 = 1.0 / math.sqrt(2.0)
TANH_CLIP = 10.0


def build_nc():
    nc = bass.Bass()

    # fp16 inputs, host-prepared layouts (see _shard_inputs)
    embN = nc.dram_tensor("embN", [128, NB * NCHUNK * H], F8,
                          kind="ExternalInput")     # (p,(b,c,h)) n=128c+p
    embT = nc.dram_tensor("embT", [128, NB * N], F16,
                          kind="ExternalInput")     # (h,(b,n))
    embF = nc.dram_tensor("embF", [NB * N, H], F16,
                          kind="ExternalInput")     # flat n-major (gather)
    distsF = nc.dram_tensor("distsF", [NB * N, NDPAD], F16,
                            kind="ExternalInput")   # pre-scaled by 1/sqrt(2)
    maskP = nc.dram_tensor("maskP", [128, NPAIR * N], F8,
                           kind="ExternalInput")    # (p2,(pr,n)) {0,-96}
    maskT = nc.dram_tensor("maskT", [128, NB * NCHUNK * 65], F8,
                           kind="ExternalInput")    # (p,(b,c,q)) q<64: vis01
    eq1T = nc.dram_tensor("eq1T", [128, NPAIR * 128], F16,
                          kind="ExternalInput")     # (h,(pr,p2))
    idxt = nc.dram_tensor("idxt", [128, NPAIR], I32,
                          kind="ExternalInput")     # flat row idx +1000*b
    loadv = nc.dram_tensor("loadv", [1, NPAIR * 128], F16,
                           kind="ExternalInput")
    wcat = nc.dram_tensor("wcat", [H, 640], F16,
                          kind="ExternalInput")     # Wf|Wl|Wg|Wv|wld(row0)
    bldT = nc.dram_tensor("bldT", [H, 1], F32,
                          kind="ExternalInput")     # b_load / sqrt(H)
    probs = nc.dram_tensor("probs", [128, NPAIR * N], F16,
                           kind="ExternalOutput")   # (p2,(pr,n))

    with tile.TileContext(nc) as tc:
        with ExitStack() as ctx:
            const = ctx.enter_context(tc.tile_pool(name="const", bufs=1))
            sb = ctx.enter_context(tc.tile_pool(name="sb", bufs=1))
            ps_big = ctx.enter_context(
                tc.tile_pool(name="ps_big", bufs=4, space="PSUM"))
            ps_small = ctx.enter_context(
                tc.tile_pool(name="ps_small", bufs=2, space="PSUM"))

            # ---- constants ----
            ident = const.tile([128, 128], F16, tag="ident")
            make_identity(nc, ident[:])
            ident8 = const.tile([128, 128], F8, tag="ident8")
            make_identity(nc, ident8[:])

            idx_s = const.tile([128, NPAIR], I32, tag="idx_s")
            nc.sync.dma_start(idx_s[:], idxt[:])
            wcat_s = const.tile([H, 640], F16, tag="wcat_s")
            nc.sync.dma_start(wcat_s[:], wcat[:])
            bld_s = const.tile([H, 1], F32, tag="bld_s")
            nc.sync.dma_start(bld_s[:], bldT[:])
            loadv_s = const.tile([1, NPAIR, 128], F16, tag="loadv_s")
            nc.sync.dma_start(loadv_s[:], loadv[:].rearrange(
                "o (q p) -> o q p", q=NPAIR))
            eq1T_s = const.tile([128, NPAIR, 128], F16, tag="eq1T_s")
            nc.sync.dma_start(eq1T_s[:], eq1T[:].rearrange(
                "h (q p) -> h q p", q=NPAIR))

            # ---- all gathers up front (gpsimd dispatch; depend on idx).
            # The indirect ring serves rows serially in dispatch order, so
            # interleave per pair: each pair's rows land before the next's.
            lastemb = sb.tile([128, NPAIR, H], F16, tag="lastemb")
            for pr in range(NPAIR):
                nc.gpsimd.indirect_dma_start(
                    out=lastemb[:, pr, :], out_offset=None, in_=embF[:],
                    in_offset=bass.IndirectOffsetOnAxis(
                        ap=idx_s[:, pr:pr + 1], axis=0))
            distg = []
            for pr in range(NPAIR):
                dg = sb.tile([128, NDPAD], F16, tag=f"distg{pr}")
                nc.gpsimd.indirect_dma_start(
                    out=dg[:], out_offset=None, in_=distsF[:],
                    in_offset=bass.IndirectOffsetOnAxis(
                        ap=idx_s[:, pr:pr + 1], axis=0))
                distg.append(dg)

            # ---- all big loads up front (sync dispatch, no waits) ----
            embNs, maskTs, embTs, mps = [], [], [], []
            for pr in range(NPAIR):
                en = sb.tile([128, 2, NCHUNK, H], F8, tag=f"embN{pr}")
                nc.sync.dma_start(en[:], embN[
                    :, pr * 2 * NCHUNK * H:(pr + 1) * 2 * NCHUNK * H]
                    .rearrange("p (b c h) -> p b c h", b=2, c=NCHUNK))
                embNs.append(en)
                mt = sb.tile([128, 2, NCHUNK, 65], F8, tag=f"maskT{pr}")
                nc.sync.dma_start(mt[:], maskT[
                    :, pr * 2 * NCHUNK * 65:(pr + 1) * 2 * NCHUNK * 65]
                    .rearrange("p (b c q) -> p b c q", b=2, c=NCHUNK))
                maskTs.append(mt)
                et = sb.tile([128, 2, N], F16, tag=f"embT{pr}")
                eng = nc.sync if pr < 2 else nc.scalar
                eng.dma_start(et[:], embT[
                    :, pr * 2 * N:(pr + 1) * 2 * N]
                    .rearrange("h (b n) -> h b n", b=2))
                embTs.append(et)
                mp = sb.tile([128, N], F8, tag=f"maskP{pr}")
                nc.scalar.dma_start(mp[:], maskP[:, pr * N:(pr + 1) * N])
                mps.append(mp)

            # ---- per-pair compute: staged software pipeline ----
            # A: qv matmuls + evicts, lastemb^T, emb^T (PE transposes)
            # B: final_q matmuls + fqT evict
            # C: score + softmax + store
            qvm_t, meanrep_t, lastembT_t, fqT_t = {}, {}, {}, {}

            def stage_A(pr):
                # q_visited pre + mean(emb): psum [h, 64+1] per batch
                qvm = sb.tile([128, 2, 64], F16, tag=f"qvm{pr}", name="qvm")
                meanrep = sb.tile([128, 128], F16, tag=f"meanrep{pr}",
                                  name="meanrep")
                for j in range(2):
                    pqv = ps_small.tile([128, 65], F32, tag="pqv", name="pqv")
                    for c in range(NCHUNK):
                        nc.tensor.matmul(
                            pqv[:],
                            lhsT=embNs[pr][:, j, c, :],
                            rhs=maskTs[pr][:, j, c, :],
                            start=(c == 0), stop=(c == NCHUNK - 1))
                    nc.vector.tensor_scalar(
                        out=qvm[:, j, :], in0=pqv[:, 0:64],
                        scalar1=INV_N, scalar2=None, op0=OP.mult)
                    nc.vector.tensor_scalar(
                        out=meanrep[:, 64 * j:64 * j + 64],
                        in0=pqv[:, 64:65].to_broadcast([128, 64]),
                        scalar1=INV_N, scalar2=None, op0=OP.mult)
                qvm_t[pr], meanrep_t[pr] = qvm, meanrep

                # lastemb^T via PE
                psl = ps_small.tile([128, 128], F16, tag="psL", bufs=1,
                                    name="psl")
                nc.tensor.transpose(out=psl[:], in_=lastemb[:, pr, :],
                                    identity=ident[:])
                lastembT = sb.tile([128, 128], F16, tag=f"lastembT{pr}",
                                   name="lastembT")
                nc.vector.tensor_copy(out=lastembT[:], in_=psl[:])
                lastembT_t[pr] = lastembT

            def stage_B(pr):
                pfq = ps_small.tile([128, 128], F32, tag="pfq", bufs=1,
                                    name="pfq")
                nc.tensor.matmul(pfq[:], lhsT=wcat_s[:, 0:128],
                                 rhs=eq1T_s[:, pr, :], start=True, stop=False)
                nc.tensor.matmul(pfq[:], lhsT=wcat_s[:, 128:256],
                                 rhs=lastembT_t[pr][:], start=False,
                                 stop=False)
                nc.tensor.matmul(pfq[:], lhsT=wcat_s[:, 256:384],
                                 rhs=meanrep_t[pr][:], start=False, stop=False)
                nc.tensor.matmul(pfq[:], lhsT=wcat_s[:, 384:512],
                                 rhs=qvm_t[pr][:], start=False, stop=False)
                nc.tensor.matmul(pfq[:], lhsT=wcat_s[0:1, 512:640],
                                 rhs=loadv_s[0:1, pr, :],
                                 start=False, stop=True)
                # fqT = psum/sqrt(H) + b_load/sqrt(H)
                fqT = sb.tile([128, 128], F16, tag=f"fqT{pr}", name="fqT")
                nc.vector.scalar_tensor_tensor(
                    out=fqT[:], in0=pfq[:], scalar=FQ2,
                    in1=bld_s[:, 0:1].to_broadcast([128, 128]),
                    op0=OP.mult, op1=OP.add)
                fqT_t[pr] = fqT

            pout = sb.tile([128, NPAIR, N], F16, tag="pout")

            def stage_C(pr):
                # score psum also absorbs -dist/sqrt2 (lhsT=-I) and the
                # {0,-1000} visited bias (lhsT=I): tanh saturates to -1 and
                # exp(-10) ~ 4.5e-5, well under the accuracy gate, so the
                # mask can ride inside the tanh argument.
                t = sb.tile([128, N], F16, tag=f"t{pr}", name="t")
                for (n0, n1) in ((0, 512), (512, N)):
                    psc = ps_big.tile([128, n1 - n0], F32, tag="psc",
                                      name="psc")
                    for j in range(2):
                        nc.tensor.matmul(
                            psc[64 * j:64 * j + 64, :],
                            lhsT=fqT_t[pr][:, 64 * j:64 * j + 64],
                            rhs=embTs[pr][:, j, n0:n1],
                            start=True, stop=False, skip_group_check=True)
                    nc.tensor.matmul(
                        psc[:], lhsT=ident[:], rhs=distg[pr][:, n0:n1],
                        start=False, stop=False, skip_group_check=True)
                    nc.tensor.matmul(
                        psc[:], lhsT=ident8[:], rhs=mps[pr][:, n0:n1],
                        start=False, stop=True, skip_group_check=True)
                    nc.scalar.activation(t[:, n0:n1], psc[:], AF.Tanh)

                e = sb.tile([128, N], F16, tag=f"e{pr}", name="e")
                ssum = sb.tile([128, 1], F32, tag=f"ssum{pr}", name="ssum")
                nc.scalar.activation(e[:], t[:], AF.Exp, scale=TANH_CLIP,
                                     accum_out=ssum[:])
                rec = sb.tile([128, 1], F32, tag=f"rec{pr}", name="rec")
                nc.vector.reciprocal(out=rec[:], in_=ssum[:])
                nc.vector.tensor_scalar(out=pout[:, pr, :], in0=e[:],
                                        scalar1=rec[:, 0:1], scalar2=None,
                                        op0=OP.mult)
                if pr % 2 == 1:
                    nc.sync.dma_start(
                        probs[:, (pr - 1) * N:(pr + 1) * N],
                        pout[:, pr - 1:pr + 1, :])

            stage_A(0)
            for pr in range(NPAIR):
                stage_B(pr)
                if pr + 1 < NPAIR:
                    stage_A(pr + 1)
                stage_C(pr)

    return nc


_CACHE = {}


def _get_nc():
    if "nc" not in _CACHE:
        _CACHE["nc"] = build_nc()
    return _CACHE["nc"]


def _shard_inputs(inputs):
    f16 = np.float16
    f8 = ml_dtypes.float8_e4m3
    dists = np.asarray(inputs["dists"], dtype=np.float32)
    embeddings = np.asarray(inputs["embeddings"], dtype=np.float32)
    encoded_q1 = np.asarray(inputs["encoded_q1"], dtype=np.float32)
    last_node = np.asarray(inputs["last_node"]).astype(np.int64)
    load = np.asarray(inputs["load"], dtype=np.float32)
    mask = np.asarray(inputs["group_ninf_mask"], dtype=np.float32)
    vis_all = (np.isneginf(mask) | (mask < -1e30))

    wcat = np.zeros((H, 640), f16)
    wcat[:, 0:128] = inputs["Wq_first"].astype(f16)
    wcat[:, 128:256] = inputs["Wq_last"].astype(f16)
    wcat[:, 256:384] = inputs["Wq_graph"].astype(f16)
    wcat[:, 384:512] = inputs["W_visited"].astype(f16)
    wcat[0, 512:640] = inputs["W_load"].astype(f16)
    bldT = (np.asarray(inputs["b_load"], dtype=np.float32) * FQ2) \
        .astype(np.float32).reshape(H, 1)

    in_maps = []
    for c in range(NCORES):
        s = slice(c * NB, (c + 1) * NB)
        emb = embeddings[s]                          # [8,1000,128]
        embT = np.ascontiguousarray(
            emb.transpose(2, 0, 1)).astype(f16).reshape(128, NB * N)
        embp = np.zeros((NB, NPAD, H), f8)
        embp[:, :N] = emb.astype(f8)
        embN = np.ascontiguousarray(
            embp.reshape(NB, NCHUNK, 128, H).transpose(2, 0, 1, 3)
        ).reshape(128, NB * NCHUNK * H)
        embF = np.ascontiguousarray(emb.reshape(NB * N, H).astype(f16))
        distsF = np.zeros((NB * N, NDPAD), f16)
        distsF[:, :N] = (dists[s].reshape(NB * N, N)
                         * np.float32(-INV_SQRT2)).astype(f16)

        vis = vis_all[s]                             # [8,64,1000] bool
        maskP = np.ascontiguousarray(
            (vis.reshape(NPAIR, 128, N).transpose(1, 0, 2))
            .astype(np.float32) * np.float32(MASK_NEG)
        ).astype(f8).reshape(128, NPAIR * N)
        visp = np.zeros((NB, NPAD, P), f8)
        visp[:, :N] = vis.transpose(0, 2, 1).astype(f8)
        maskT = np.concatenate(
            [visp.reshape(NB, NCHUNK, 128, P).transpose(2, 0, 1, 3),
             np.ones((128, NB, NCHUNK, 1), f8)],
            axis=3).reshape(128, NB * NCHUNK * 65)
        maskT = np.ascontiguousarray(maskT)

        eq1T = np.ascontiguousarray(
            encoded_q1[s].astype(f16).transpose(2, 0, 1)
        ).reshape(128, NPAIR * 128)
        idxt = np.ascontiguousarray(
            (last_node[s] + np.arange(NB)[:, None] * N)
            .astype(np.int32).reshape(NPAIR, 128).T)
        loadv = load[s].astype(f16).reshape(1, NPAIR * 128)

        in_maps.append(dict(
            embN=embN, embT=embT, embF=embF, distsF=distsF,
            maskP=maskP, maskT=maskT, eq1T=eq1T, idxt=idxt,
            loadv=loadv, wcat=wcat, bldT=bldT,
        ))
    return in_maps


def _run(inputs, trace=False, **kw):
    nc = _get_nc()
    in_maps = _shard_inputs(inputs)
    res = run_bass_kernel_spmd(nc, in_maps, list(range(NCORES)),
                               trace=trace, **kw)
    out = np.concatenate(
        [r["probs"].astype(np.float32).reshape(128, NPAIR, N)
         .transpose(1, 0, 2).reshape(NB, P, N)
         for r in res.results], axis=0)
    return out, res


def kernel(**inputs) -> np.ndarray:
    out, _ = _run(inputs)
    return out


# revision 37
# speedup vs baseline: 1.0989x; 1.0502x over previous
"""CVRP decoder kernel for Trainium2 (8 NeuronCores, batch-data-parallel).

Computes, per batch b (B=64, P=64, N=1000, H=128):
    q_graph   = mean_n(emb) @ Wq_graph
    q_first   = encoded_q1 @ Wq_first
    q_last    = emb[last_node] @ Wq_last
    q_visited = (vis01 @ emb / N) @ W_visited          (vis01 = isneginf(mask))
    final_q   = sum of the above + load*W_load + b_load
    score     = final_q @ emb^T / sqrt(H) - dists[last_node] / sqrt(2)
    probs     = softmax(10*tanh(score) + (-BIG if visited))

Sharding: batch across the 8 cores (pure data parallel); the 8 batches per
core run as 4 pairs of 2 batches stacked on the 128 SBUF partitions.

Device kernel design (per core):
  * All matmul operands 16/8-bit, fp32 PSUM: emb n-major + visited mask
    in fp8e4m3 (qv/mean matmuls), emb h-major + final_q in fp16 (score).
  * The dist bias and the additive visited mask are folded into the score
    PSUM with identity-lhsT matmuls, so tanh reads PSUM directly and the
    whole mask/bias elementwise stage disappears.  The mask rides INSIDE
    the tanh argument (-96 saturates tanh to -1; exp(-10)=4.5e-5 is far
    below the accuracy gate).
  * dists rows are gathered on-device by last_node (indirect DMA, rows
    padded >2048B so they ride the software DGE ring which overlaps the
    bulk loads; the hardware-dynamic indirect ring is starved until all
    direct loads drain).
  * Loads are split across the two HWDGE dispatch rings (sync + scalar
    engines) plus the gpsimd software ring, all issued up front with no
    per-pair store-waits blocking later loads.
  * Stages are software-pipelined across pairs (A=qv/mean/lastemb^T,
    B=final_q, C=score+softmax+store) so each engine's small ops are
    queued ahead of the next pair's long activations.

Host-side prep inside kernel() (plain numpy, layout/dtype only): fp16/fp8
casts, transposed layouts (emb^T, mask^T, eq1^T), flat gather indices,
constant folding (-dists/sqrt2, b_load/sqrt(H)); all matmuls, gathers and
the softmax run on device.
"""

import json
import math
import numpy as np
import ml_dtypes
from contextlib import ExitStack

import concourse.bass as bass
import concourse.mybir as mybir
import concourse.tile as tile
from concourse.bass_utils import run_bass_kernel_spmd
from concourse.masks import make_identity


def _split_excess_waits(bir_bytes: bytes, max_waits: int = 1) -> bytes:
    """Walrus in this image rejects instructions carrying too many sem waits
    ("Too many sync wait commands", e.g. on Tile's kernel-tail Drain).
    Hoist excess waits onto preceding same-engine EventSemaphore carriers
    (pure sync ops) — sems are monotonic, so a chain of instructions whose
    waits partition the original list is equivalent."""
    d = json.loads(bir_bytes)
    n = [0]
    for fn in d.get("functions", []):
        for blk in fn.get("blocks", []):
            out = []
            for ins in blk.get("instructions", []):
                si = ins.get("sync_info") or {}
                waits = si.get("on_wait") or []
                if len(waits) > max_waits:
                    extra, keep = waits[:-max_waits], waits[-max_waits:]
                    ins["sync_info"]["on_wait"] = keep
                    for i in range(0, len(extra), max_waits):
                        n[0] += 1
                        carrier = {
                            "name": f"I-waitsplit-{n[0]}",
                            "opcode": "EventSemaphore",
                            "engine": ins["engine"],
                            "ins": [],
                            "outs": [],
                            "sync_info": {
                                "on_update": [],
                                "on_wait": extra[i:i + max_waits],
                            },
                        }
                        if "debug" in ins:
                            carrier["debug"] = ins["debug"]
                        out.append(carrier)
                out.append(ins)
            blk["instructions"] = out
    return json.dumps(d).encode()


def _install_walrus_shim():
    import concourse.bass2jax as b2j
    import concourse.bass_utils as bu
    if getattr(bu, "_waitsplit_installed", False):
        return
    real = bu.compile_bir_kernel

    def patched(bir_json, tmpdir, neff_name="file.neff", **kw):
        if isinstance(bir_json, (bytes, bytearray, str)):
            if isinstance(bir_json, str):
                bir_json = bir_json.encode()
            bir_json = _split_excess_waits(bir_json)
        return real(bir_json, tmpdir, neff_name=neff_name, **kw)

    bu.compile_bir_kernel = patched
    b2j.compile_bir_kernel = patched
    bu._waitsplit_installed = True


_install_walrus_shim()

F32 = mybir.dt.float32
F8 = mybir.dt.float8e4
F16 = mybir.dt.float16
I32 = mybir.dt.int32
OP = mybir.AluOpType
AF = mybir.ActivationFunctionType

B, P, N, H = 64, 64, 1000, 128
NCORES = 8
NB = B // NCORES          # 8 batches per core
NPAIR = NB // 2           # 4 pairs
NCHUNK = 8                # 8 n-chunks of 128 (n padded 1000 -> 1024)
NPAD = 1024
NDPAD = 1088              # dists rows padded >2048B so the gather rides SWDGE

MASK_NEG = -96.0          # visited bias inside tanh arg (saturates tanh to -1);
                          # exactly representable in fp8e4m3
INV_N = 1.0 / N
FQ2 = 1.0 / math.sqrt(H)
INV_SQRT2 = 1.0 / math.sqrt(2.0)
TANH_CLIP = 10.0


def build_nc():
    nc = bass.Bass()

    # fp16 inputs, host-prepared layouts (see _shard_inputs)
    embN = nc.dram_tensor("embN", [128, NB * NCHUNK * H], F8,
                          kind="ExternalInput")     # (p,(b,c,h)) n=128c+p
    embT = nc.dram_tensor("embT", [128, NB * N], F16,
                          kind="ExternalInput")     # (h,(b,n))
    embF = nc.dram_tensor("embF", [NB * N, H], F16,
                          kind="ExternalInput")     # flat n-major (gather)
    distsF = nc.dram_tensor("distsF", [NB * N, NDPAD], F16,
                            kind="ExternalInput")   # pre-scaled by 1/sqrt(2)
    maskP = nc.dram_tensor("maskP", [128, NPAIR * N], F8,
                           kind="ExternalInput")    # (p2,(pr,n)) {0,-96}
    maskT = nc.dram_tensor("maskT", [128, NB * NCHUNK * 65], F8,
                           kind="ExternalInput")    # (p,(b,c,q)) q<64: vis01
    eq1T = nc.dram_tensor("eq1T", [128, NPAIR * 128], F16,
                          kind="ExternalInput")     # (h,(pr,p2))
    idxt = nc.dram_tensor("idxt", [128, NPAIR], I32,
                          kind="ExternalInput")     # flat row idx +1000*b
    loadv = nc.dram_tensor("loadv", [1, NPAIR * 128], F16,
                           kind="ExternalInput")
    wcat = nc.dram_tensor("wcat", [H, 640], F16,
                          kind="ExternalInput")     # Wf|Wl|Wg|Wv|wld(row0)
    bldT = nc.dram_tensor("bldT", [H, 1], F32,
                          kind="ExternalInput")     # b_load / sqrt(H)
    probs = nc.dram_tensor("probs", [128, NPAIR * N], F16,
                           kind="ExternalOutput")   # (p2,(pr,n))

    with tile.TileContext(nc) as tc:
        with ExitStack() as ctx:
            const = ctx.enter_context(tc.tile_pool(name="const", bufs=1))
            sb = ctx.enter_context(tc.tile_pool(name="sb", bufs=1))
            ps_big = ctx.enter_context(
                tc.tile_pool(name="ps_big", bufs=4, space="PSUM"))
            ps_small = ctx.enter_context(
                tc.tile_pool(name="ps_small", bufs=2, space="PSUM"))

            # ---- constants ----
            ident = const.tile([128, 128], F16, tag="ident")
            make_identity(nc, ident[:])
            ident8 = const.tile([128, 128], F8, tag="ident8")
            make_identity(nc, ident8[:])

            idx_s = const.tile([128, NPAIR], I32, tag="idx_s")
            nc.sync.dma_start(idx_s[:], idxt[:])
            wcat_s = const.tile([H, 640], F16, tag="wcat_s")
            nc.sync.dma_start(wcat_s[:], wcat[:])
            bld_s = const.tile([H, 1], F32, tag="bld_s")
            nc.sync.dma_start(bld_s[:], bldT[:])
            loadv_s = const.tile([1, NPAIR, 128], F16, tag="loadv_s")
            nc.sync.dma_start(loadv_s[:], loadv[:].rearrange(
                "o (q p) -> o q p", q=NPAIR))
            eq1T_s = const.tile([128, NPAIR, 128], F16, tag="eq1T_s")
            nc.sync.dma_start(eq1T_s[:], eq1T[:].rearrange(
                "h (q p) -> h q p", q=NPAIR))

            # ---- all gathers up front (gpsimd dispatch; depend on idx).
            # The indirect ring serves rows serially in dispatch order, so
            # interleave per pair: each pair's rows land before the next's.
            lastemb = sb.tile([128, NPAIR, H], F16, tag="lastemb")
            for pr in range(NPAIR):
                nc.gpsimd.indirect_dma_start(
                    out=lastemb[:, pr, :], out_offset=None, in_=embF[:],
                    in_offset=bass.IndirectOffsetOnAxis(
                        ap=idx_s[:, pr:pr + 1], axis=0))
            distg = []
            for pr in range(NPAIR):
                dg = sb.tile([128, NDPAD], F16, tag=f"distg{pr}")
                nc.gpsimd.indirect_dma_start(
                    out=dg[:], out_offset=None, in_=distsF[:],
                    in_offset=bass.IndirectOffsetOnAxis(
                        ap=idx_s[:, pr:pr + 1], axis=0))
                distg.append(dg)

            # ---- all big loads up front (sync dispatch, no waits) ----
            embNs, maskTs, embTs, mps = [], [], [], []
            for pr in range(NPAIR):
                en = sb.tile([128, 2, NCHUNK, H], F8, tag=f"embN{pr}")
                nc.sync.dma_start(en[:], embN[
                    :, pr * 2 * NCHUNK * H:(pr + 1) * 2 * NCHUNK * H]
                    .rearrange("p (b c h) -> p b c h", b=2, c=NCHUNK))
                embNs.append(en)
                mt = sb.tile([128, 2, NCHUNK, 65], F8, tag=f"maskT{pr}")
                nc.sync.dma_start(mt[:], maskT[
                    :, pr * 2 * NCHUNK * 65:(pr + 1) * 2 * NCHUNK * 65]
                    .rearrange("p (b c q) -> p b c q", b=2, c=NCHUNK))
                maskTs.append(mt)
                et = sb.tile([128, 2, N], F16, tag=f"embT{pr}")
                eng = nc.sync if pr < 2 else nc.scalar
                eng.dma_start(et[:], embT[
                    :, pr * 2 * N:(pr + 1) * 2 * N]
                    .rearrange("h (b n) -> h b n", b=2))
                embTs.append(et)
                mp = sb.tile([128, N], F8, tag=f"maskP{pr}")
                nc.scalar.dma_start(mp[:], maskP[:, pr * N:(pr + 1) * N])
                mps.append(mp)

            # ---- per-pair compute: staged software pipeline ----
            # A: qv matmuls + evicts, lastemb^T, emb^T (PE transposes)
            # B: final_q matmuls + fqT evict
            # C: score + softmax + store
            qvm_t, meanrep_t, lastembT_t, fqT_t = {}, {}, {}, {}

            def stage_A(pr):
                # q_visited pre + mean(emb): psum [h, 64+1] per batch
                qvm = sb.tile([128, 2, 64], F16, tag=f"qvm{pr}", name="qvm")
                meanrep = sb.tile([128, 128], F16, tag=f"meanrep{pr}",
                                  name="meanrep")
                for j in range(2):
                    pqv = ps_small.tile([128, 65], F32, tag="pqv", name="pqv")
                    for c in range(NCHUNK):
                        nc.tensor.matmul(
                            pqv[:],
                            lhsT=embNs[pr][:, j, c, :],
                            rhs=maskTs[pr][:, j, c, :],
                            start=(c == 0), stop=(c == NCHUNK - 1))
                    nc.vector.tensor_scalar(
                        out=qvm[:, j, :], in0=pqv[:, 0:64],
                        scalar1=INV_N, scalar2=None, op0=OP.mult)
                    nc.vector.tensor_scalar(
                        out=meanrep[:, 64 * j:64 * j + 64],
                        in0=pqv[:, 64:65].to_broadcast([128, 64]),
                        scalar1=INV_N, scalar2=None, op0=OP.mult)
                qvm_t[pr], meanrep_t[pr] = qvm, meanrep

                # lastemb^T via PE
                psl = ps_small.tile([128, 128], F16, tag="psL", bufs=1,
                                    name="psl")
                nc.tensor.transpose(out=psl[:], in_=lastemb[:, pr, :],
                                    identity=ident[:])
                lastembT = sb.tile([128, 128], F16, tag=f"lastembT{pr}",
                                   name="lastembT")
                nc.vector.tensor_copy(out=lastembT[:], in_=psl[:])
                lastembT_t[pr] = lastembT

            def stage_B(pr):
                pfq = ps_small.tile([128, 128], F32, tag="pfq", bufs=1,
                                    name="pfq")
                nc.tensor.matmul(pfq[:], lhsT=wcat_s[:, 0:128],
                                 rhs=eq1T_s[:, pr, :], start=True, stop=False)
                nc.tensor.matmul(pfq[:], lhsT=wcat_s[:, 128:256],
                                 rhs=lastembT_t[pr][:], start=False,
                                 stop=False)
                nc.tensor.matmul(pfq[:], lhsT=wcat_s[:, 256:384],
                                 rhs=meanrep_t[pr][:], start=False, stop=False)
                nc.tensor.matmul(pfq[:], lhsT=wcat_s[:, 384:512],
                                 rhs=qvm_t[pr][:], start=False, stop=False)
                nc.tensor.matmul(pfq[:], lhsT=wcat_s[0:1, 512:640],
                                 rhs=loadv_s[0:1, pr, :],
                                 start=False, stop=True)
                # fqT = psum/sqrt(H) + b_load/sqrt(H)
                fqT = sb.tile([128, 128], F16, tag=f"fqT{pr}", name="fqT")
                nc.vector.scalar_tensor_tensor(
                    out=fqT[:], in0=pfq[:], scalar=FQ2,
                    in1=bld_s[:, 0:1].to_broadcast([128, 128]),
                    op0=OP.mult, op1=OP.add)
                fqT_t[pr] = fqT

            pout = sb.tile([128, NPAIR, N], F16, tag="pout")

            def stage_C(pr):
                # score psum also absorbs -dist/sqrt2 (lhsT=-I) and the
                # {0,-1000} visited bias (lhsT=I): tanh saturates to -1 and
                # exp(-10) ~ 4.5e-5, well under the accuracy gate, so the
                # mask can ride inside the tanh argument.
                t = sb.tile([128, N], F16, tag=f"t{pr}", name="t")
                for (n0, n1) in ((0, 512), (512, N)):
                    psc = ps_big.tile([128, n1 - n0], F32, tag="psc",
                                      name="psc")
                    for j in range(2):
                        nc.tensor.matmul(
                            psc[64 * j:64 * j + 64, :],
                            lhsT=fqT_t[pr][:, 64 * j:64 * j + 64],
                            rhs=embTs[pr][:, j, n0:n1],
                            start=True, stop=False, skip_group_check=True)
                    nc.tensor.matmul(
                        psc[:], lhsT=ident[:], rhs=distg[pr][:, n0:n1],
                        start=False, stop=False, skip_group_check=True)
                    nc.tensor.matmul(
                        psc[:], lhsT=ident8[:], rhs=mps[pr][:, n0:n1],
                        start=False, stop=True, skip_group_check=True)
                    nc.scalar.activation(t[:, n0:n1], psc[:], AF.Tanh)

                e = sb.tile([128, N], F16, tag=f"e{pr}", name="e")
                ssum = sb.tile([128, 1], F32, tag=f"ssum{pr}", name="ssum")
                nc.scalar.activation(e[:], t[:], AF.Exp, scale=TANH_CLIP,
                                     accum_out=ssum[:])
                rec = sb.tile([128, 1], F32, tag=f"rec{pr}", name="rec")
                nc.vector.reciprocal(out=rec[:], in_=ssum[:])
                nc.vector.tensor_scalar(out=pout[:, pr, :], in0=e[:],
                                        scalar1=rec[:, 0:1], scalar2=None,
                                        op0=OP.mult)
                if pr == 1:
                    nc.sync.dma_start(
                        probs[:, 0:2 * N], pout[:, 0:2, :])
                elif pr == 2:
                    nc.sync.dma_start(
                        probs[:, 2 * N:3 * N], pout[:, 2, :])
                elif pr == 3:
                    nc.scalar.dma_start(
                        probs[:, 3 * N:4 * N], pout[:, 3, :])

            stage_A(0)
            for pr in range(NPAIR):
                stage_B(pr)
                if pr + 1 < NPAIR:
                    stage_A(pr + 1)
                stage_C(pr)

    return nc


_CACHE = {}


def _get_nc():
    if "nc" not in _CACHE:
        _CACHE["nc"] = build_nc()
    return _CACHE["nc"]


def _shard_inputs(inputs):
    f16 = np.float16
    f8 = ml_dtypes.float8_e4m3
    dists = np.asarray(inputs["dists"], dtype=np.float32)
    embeddings = np.asarray(inputs["embeddings"], dtype=np.float32)
    encoded_q1 = np.asarray(inputs["encoded_q1"], dtype=np.float32)
    last_node = np.asarray(inputs["last_node"]).astype(np.int64)
    load = np.asarray(inputs["load"], dtype=np.float32)
    mask = np.asarray(inputs["group_ninf_mask"], dtype=np.float32)
    vis_all = (np.isneginf(mask) | (mask < -1e30))

    wcat = np.zeros((H, 640), f16)
    wcat[:, 0:128] = inputs["Wq_first"].astype(f16)
    wcat[:, 128:256] = inputs["Wq_last"].astype(f16)
    wcat[:, 256:384] = inputs["Wq_graph"].astype(f16)
    wcat[:, 384:512] = inputs["W_visited"].astype(f16)
    wcat[0, 512:640] = inputs["W_load"].astype(f16)
    bldT = (np.asarray(inputs["b_load"], dtype=np.float32) * FQ2) \
        .astype(np.float32).reshape(H, 1)

    in_maps = []
    for c in range(NCORES):
        s = slice(c * NB, (c + 1) * NB)
        emb = embeddings[s]                          # [8,1000,128]
        embT = np.ascontiguousarray(
            emb.transpose(2, 0, 1)).astype(f16).reshape(128, NB * N)
        embp = np.zeros((NB, NPAD, H), f8)
        embp[:, :N] = emb.astype(f8)
        embN = np.ascontiguousarray(
            embp.reshape(NB, NCHUNK, 128, H).transpose(2, 0, 1, 3)
        ).reshape(128, NB * NCHUNK * H)
        embF = np.ascontiguousarray(emb.reshape(NB * N, H).astype(f16))
        distsF = np.zeros((NB * N, NDPAD), f16)
        distsF[:, :N] = (dists[s].reshape(NB * N, N)
                         * np.float32(-INV_SQRT2)).astype(f16)

        vis = vis_all[s]                             # [8,64,1000] bool
        maskP = np.ascontiguousarray(
            (vis.reshape(NPAIR, 128, N).transpose(1, 0, 2))
            .astype(np.float32) * np.float32(MASK_NEG)
        ).astype(f8).reshape(128, NPAIR * N)
        visp = np.zeros((NB, NPAD, P), f8)
        visp[:, :N] = vis.transpose(0, 2, 1).astype(f8)
        maskT = np.concatenate(
            [visp.reshape(NB, NCHUNK, 128, P).transpose(2, 0, 1, 3),
             np.ones((128, NB, NCHUNK, 1), f8)],
            axis=3).reshape(128, NB * NCHUNK * 65)
        maskT = np.ascontiguousarray(maskT)

        eq1T = np.ascontiguousarray(
            encoded_q1[s].astype(f16).transpose(2, 0, 1)
        ).reshape(128, NPAIR * 128)
        idxt = np.ascontiguousarray(
            (last_node[s] + np.arange(NB)[:, None] * N)
            .astype(np.int32).reshape(NPAIR, 128).T)
        loadv = load[s].astype(f16).reshape(1, NPAIR * 128)

        in_maps.append(dict(
            embN=embN, embT=embT, embF=embF, distsF=distsF,
            maskP=maskP, maskT=maskT, eq1T=eq1T, idxt=idxt,
            loadv=loadv, wcat=wcat, bldT=bldT,
        ))
    return in_maps


def _run(inputs, trace=False, **kw):
    nc = _get_nc()
    in_maps = _shard_inputs(inputs)
    res = run_bass_kernel_spmd(nc, in_maps, list(range(NCORES)),
                               trace=trace, **kw)
    out = np.concatenate(
        [r["probs"].astype(np.float32).reshape(128, NPAIR, N)
         .transpose(1, 0, 2).reshape(NB, P, N)
         for r in res.results], axis=0)
    return out, res


def kernel(**inputs) -> np.ndarray:
    out, _ = _run(inputs)
    return out


# revision 38
# speedup vs baseline: 1.1008x; 1.0017x over previous
"""CVRP decoder kernel for Trainium2 (8 NeuronCores, batch-data-parallel).

Computes, per batch b (B=64, P=64, N=1000, H=128):
    q_graph   = mean_n(emb) @ Wq_graph
    q_first   = encoded_q1 @ Wq_first
    q_last    = emb[last_node] @ Wq_last
    q_visited = (vis01 @ emb / N) @ W_visited          (vis01 = isneginf(mask))
    final_q   = sum of the above + load*W_load + b_load
    score     = final_q @ emb^T / sqrt(H) - dists[last_node] / sqrt(2)
    probs     = softmax(10*tanh(score) + (-BIG if visited))

Sharding: batch across the 8 cores (pure data parallel); the 8 batches per
core run as 4 pairs of 2 batches stacked on the 128 SBUF partitions.

Device kernel design (per core):
  * All matmul operands 16/8-bit, fp32 PSUM: emb n-major + visited mask
    in fp8e4m3 (qv/mean matmuls), emb h-major + final_q in fp16 (score).
  * The dist bias and the additive visited mask are folded into the score
    PSUM with identity-lhsT matmuls, so tanh reads PSUM directly and the
    whole mask/bias elementwise stage disappears.  The mask rides INSIDE
    the tanh argument (-96 saturates tanh to -1; exp(-10)=4.5e-5 is far
    below the accuracy gate).
  * dists rows are gathered on-device by last_node (indirect DMA, rows
    padded >2048B so they ride the software DGE ring which overlaps the
    bulk loads; the hardware-dynamic indirect ring is starved until all
    direct loads drain).
  * Loads are split across the two HWDGE dispatch rings (sync + scalar
    engines) plus the gpsimd software ring, all issued up front with no
    per-pair store-waits blocking later loads.  The final two stores are
    split across BOTH ring-sets (pair 2 on sync, pair 3 on scalar) so the
    last 0.25MB store transfer is halved and pair 2's store overlaps
    pair 3's softmax.
  * Stages are software-pipelined across pairs (A=qv/mean/lastemb^T,
    B=final_q, C=score+softmax+store) so each engine's small ops are
    queued ahead of the next pair's long activations.

Host-side prep inside kernel() (plain numpy, layout/dtype only): fp16/fp8
casts, transposed layouts (emb^T, mask^T, eq1^T), flat gather indices,
constant folding (-dists/sqrt2, b_load/sqrt(H)); all matmuls, gathers and
the softmax run on device.
"""

import json
import math
import numpy as np
import ml_dtypes
from contextlib import ExitStack

import concourse.bass as bass
import concourse.mybir as mybir
import concourse.tile as tile
from concourse.bass_utils import run_bass_kernel_spmd
from concourse.masks import make_identity


def _split_excess_waits(bir_bytes: bytes, max_waits: int = 1) -> bytes:
    """Walrus in this image rejects instructions carrying too many sem waits
    ("Too many sync wait commands", e.g. on Tile's kernel-tail Drain).
    Hoist excess waits onto preceding same-engine EventSemaphore carriers
    (pure sync ops) — sems are monotonic, so a chain of instructions whose
    waits partition the original list is equivalent."""
    d = json.loads(bir_bytes)
    n = [0]
    for fn in d.get("functions", []):
        for blk in fn.get("blocks", []):
            out = []
            for ins in blk.get("instructions", []):
                si = ins.get("sync_info") or {}
                waits = si.get("on_wait") or []
                if len(waits) > max_waits:
                    extra, keep = waits[:-max_waits], waits[-max_waits:]
                    ins["sync_info"]["on_wait"] = keep
                    for i in range(0, len(extra), max_waits):
                        n[0] += 1
                        carrier = {
                            "name": f"I-waitsplit-{n[0]}",
                            "opcode": "EventSemaphore",
                            "engine": ins["engine"],
                            "ins": [],
                            "outs": [],
                            "sync_info": {
                                "on_update": [],
                                "on_wait": extra[i:i + max_waits],
                            },
                        }
                        if "debug" in ins:
                            carrier["debug"] = ins["debug"]
                        out.append(carrier)
                out.append(ins)
            blk["instructions"] = out
    return json.dumps(d).encode()


def _install_walrus_shim():
    import concourse.bass2jax as b2j
    import concourse.bass_utils as bu
    if getattr(bu, "_waitsplit_installed", False):
        return
    real = bu.compile_bir_kernel

    def patched(bir_json, tmpdir, neff_name="file.neff", **kw):
        if isinstance(bir_json, (bytes, bytearray, str)):
            if isinstance(bir_json, str):
                bir_json = bir_json.encode()
            bir_json = _split_excess_waits(bir_json)
        return real(bir_json, tmpdir, neff_name=neff_name, **kw)

    bu.compile_bir_kernel = patched
    b2j.compile_bir_kernel = patched
    bu._waitsplit_installed = True


_install_walrus_shim()

F32 = mybir.dt.float32
F8 = mybir.dt.float8e4
F16 = mybir.dt.float16
I32 = mybir.dt.int32
OP = mybir.AluOpType
AF = mybir.ActivationFunctionType

B, P, N, H = 64, 64, 1000, 128
NCORES = 8
NB = B // NCORES          # 8 batches per core
NPAIR = NB // 2           # 4 pairs
NCHUNK = 8                # 8 n-chunks of 128 (n padded 1000 -> 1024)
NPAD = 1024
NDPAD = 1088              # dists rows padded >2048B so the gather rides SWDGE

MASK_NEG = -96.0          # visited bias inside tanh arg (saturates tanh to -1);
                          # exactly representable in fp8e4m3
INV_N = 1.0 / N
FQ2 = 1.0 / math.sqrt(H)
INV_SQRT2 = 1.0 / math.sqrt(2.0)
TANH_CLIP = 10.0


def build_nc():
    nc = bass.Bass()

    # fp16 inputs, host-prepared layouts (see _shard_inputs)
    embN = nc.dram_tensor("embN", [128, NB * NCHUNK * H], F8,
                          kind="ExternalInput")     # (p,(b,c,h)) n=128c+p
    embT = nc.dram_tensor("embT", [128, NB * N], F16,
                          kind="ExternalInput")     # (h,(b,n))
    embF = nc.dram_tensor("embF", [NB * N, H], F16,
                          kind="ExternalInput")     # flat n-major (gather)
    distsF = nc.dram_tensor("distsF", [NB * N, NDPAD], F16,
                            kind="ExternalInput")   # pre-scaled by 1/sqrt(2)
    maskP = nc.dram_tensor("maskP", [128, NPAIR * N], F8,
                           kind="ExternalInput")    # (p2,(pr,n)) {0,-96}
    maskT = nc.dram_tensor("maskT", [128, NB * NCHUNK * 65], F8,
                           kind="ExternalInput")    # (p,(b,c,q)) q<64: vis01
    eq1T = nc.dram_tensor("eq1T", [128, NPAIR * 128], F16,
                          kind="ExternalInput")     # (h,(pr,p2))
    idxt = nc.dram_tensor("idxt", [128, NPAIR], I32,
                          kind="ExternalInput")     # flat row idx +1000*b
    loadv = nc.dram_tensor("loadv", [1, NPAIR * 128], F16,
                           kind="ExternalInput")
    wcat = nc.dram_tensor("wcat", [H, 640], F16,
                          kind="ExternalInput")     # Wf|Wl|Wg|Wv|wld(row0)
    bldT = nc.dram_tensor("bldT", [H, 1], F32,
                          kind="ExternalInput")     # b_load / sqrt(H)
    probs = nc.dram_tensor("probs", [128, NPAIR * N], F16,
                           kind="ExternalOutput")   # (p2,(pr,n))

    with tile.TileContext(nc) as tc:
        with ExitStack() as ctx:
            const = ctx.enter_context(tc.tile_pool(name="const", bufs=1))
            sb = ctx.enter_context(tc.tile_pool(name="sb", bufs=1))
            ps_big = ctx.enter_context(
                tc.tile_pool(name="ps_big", bufs=4, space="PSUM"))
            ps_small = ctx.enter_context(
                tc.tile_pool(name="ps_small", bufs=2, space="PSUM"))

            # ---- constants ----
            ident = const.tile([128, 128], F16, tag="ident")
            make_identity(nc, ident[:])
            ident8 = const.tile([128, 128], F8, tag="ident8")
            make_identity(nc, ident8[:])

            idx_s = const.tile([128, NPAIR], I32, tag="idx_s")
            nc.sync.dma_start(idx_s[:], idxt[:])
            wcat_s = const.tile([H, 640], F16, tag="wcat_s")
            nc.sync.dma_start(wcat_s[:], wcat[:])
            bld_s = const.tile([H, 1], F32, tag="bld_s")
            nc.sync.dma_start(bld_s[:], bldT[:])
            loadv_s = const.tile([1, NPAIR, 128], F16, tag="loadv_s")
            nc.sync.dma_start(loadv_s[:], loadv[:].rearrange(
                "o (q p) -> o q p", q=NPAIR))
            eq1T_s = const.tile([128, NPAIR, 128], F16, tag="eq1T_s")
            nc.sync.dma_start(eq1T_s[:], eq1T[:].rearrange(
                "h (q p) -> h q p", q=NPAIR))

            # ---- all gathers up front (gpsimd dispatch; depend on idx).
            # The indirect ring serves rows serially in dispatch order, so
            # interleave per pair: each pair's rows land before the next's.
            lastemb = sb.tile([128, NPAIR, H], F16, tag="lastemb")
            for pr in range(NPAIR):
                nc.gpsimd.indirect_dma_start(
                    out=lastemb[:, pr, :], out_offset=None, in_=embF[:],
                    in_offset=bass.IndirectOffsetOnAxis(
                        ap=idx_s[:, pr:pr + 1], axis=0))
            distg = []
            for pr in range(NPAIR):
                dg = sb.tile([128, NDPAD], F16, tag=f"distg{pr}")
                nc.gpsimd.indirect_dma_start(
                    out=dg[:], out_offset=None, in_=distsF[:],
                    in_offset=bass.IndirectOffsetOnAxis(
                        ap=idx_s[:, pr:pr + 1], axis=0))
                distg.append(dg)

            # ---- all big loads up front (sync dispatch, no waits) ----
            embNs, maskTs, embTs, mps = [], [], [], []
            for pr in range(NPAIR):
                en = sb.tile([128, 2, NCHUNK, H], F8, tag=f"embN{pr}")
                nc.sync.dma_start(en[:], embN[
                    :, pr * 2 * NCHUNK * H:(pr + 1) * 2 * NCHUNK * H]
                    .rearrange("p (b c h) -> p b c h", b=2, c=NCHUNK))
                embNs.append(en)
                mt = sb.tile([128, 2, NCHUNK, 65], F8, tag=f"maskT{pr}")
                nc.sync.dma_start(mt[:], maskT[
                    :, pr * 2 * NCHUNK * 65:(pr + 1) * 2 * NCHUNK * 65]
                    .rearrange("p (b c q) -> p b c q", b=2, c=NCHUNK))
                maskTs.append(mt)
                et = sb.tile([128, 2, N], F16, tag=f"embT{pr}")
                eng = nc.sync if pr < 2 else nc.scalar
                eng.dma_start(et[:], embT[
                    :, pr * 2 * N:(pr + 1) * 2 * N]
                    .rearrange("h (b n) -> h b n", b=2))
                embTs.append(et)
                mp = sb.tile([128, N], F8, tag=f"maskP{pr}")
                nc.scalar.dma_start(mp[:], maskP[:, pr * N:(pr + 1) * N])
                mps.append(mp)

            # ---- per-pair compute: staged software pipeline ----
            # A: qv matmuls + evicts, lastemb^T, emb^T (PE transposes)
            # B: final_q matmuls + fqT evict
            # C: score + softmax + store
            qvm_t, meanrep_t, lastembT_t, fqT_t = {}, {}, {}, {}

            def stage_A(pr):
                # q_visited pre + mean(emb): psum [h, 64+1] per batch
                qvm = sb.tile([128, 2, 64], F16, tag=f"qvm{pr}", name="qvm")
                meanrep = sb.tile([128, 128], F16, tag=f"meanrep{pr}",
                                  name="meanrep")
                for j in range(2):
                    pqv = ps_small.tile([128, 65], F32, tag="pqv", name="pqv")
                    for c in range(NCHUNK):
                        nc.tensor.matmul(
                            pqv[:],
                            lhsT=embNs[pr][:, j, c, :],
                            rhs=maskTs[pr][:, j, c, :],
                            start=(c == 0), stop=(c == NCHUNK - 1))
                    nc.vector.tensor_scalar(
                        out=qvm[:, j, :], in0=pqv[:, 0:64],
                        scalar1=INV_N, scalar2=None, op0=OP.mult)
                    nc.vector.tensor_scalar(
                        out=meanrep[:, 64 * j:64 * j + 64],
                        in0=pqv[:, 64:65].to_broadcast([128, 64]),
                        scalar1=INV_N, scalar2=None, op0=OP.mult)
                qvm_t[pr], meanrep_t[pr] = qvm, meanrep

                # lastemb^T via PE
                psl = ps_small.tile([128, 128], F16, tag="psL", bufs=1,
                                    name="psl")
                nc.tensor.transpose(out=psl[:], in_=lastemb[:, pr, :],
                                    identity=ident[:])
                lastembT = sb.tile([128, 128], F16, tag=f"lastembT{pr}",
                                   name="lastembT")
                nc.vector.tensor_copy(out=lastembT[:], in_=psl[:])
                lastembT_t[pr] = lastembT

            def stage_B(pr):
                pfq = ps_small.tile([128, 128], F32, tag="pfq", bufs=1,
                                    name="pfq")
                nc.tensor.matmul(pfq[:], lhsT=wcat_s[:, 0:128],
                                 rhs=eq1T_s[:, pr, :], start=True, stop=False)
                nc.tensor.matmul(pfq[:], lhsT=wcat_s[:, 128:256],
                                 rhs=lastembT_t[pr][:], start=False,
                                 stop=False)
                nc.tensor.matmul(pfq[:], lhsT=wcat_s[:, 256:384],
                                 rhs=meanrep_t[pr][:], start=False, stop=False)
                nc.tensor.matmul(pfq[:], lhsT=wcat_s[:, 384:512],
                                 rhs=qvm_t[pr][:], start=False, stop=False)
                nc.tensor.matmul(pfq[:], lhsT=wcat_s[0:1, 512:640],
                                 rhs=loadv_s[0:1, pr, :],
                                 start=False, stop=True)
                # fqT = psum/sqrt(H) + b_load/sqrt(H)
                fqT = sb.tile([128, 128], F16, tag=f"fqT{pr}", name="fqT")
                nc.vector.scalar_tensor_tensor(
                    out=fqT[:], in0=pfq[:], scalar=FQ2,
                    in1=bld_s[:, 0:1].to_broadcast([128, 128]),
                    op0=OP.mult, op1=OP.add)
                fqT_t[pr] = fqT

            pout = sb.tile([128, NPAIR, N], F16, tag="pout")

            def stage_C(pr):
                # score psum also absorbs -dist/sqrt2 (lhsT=-I) and the
                # {0,-1000} visited bias (lhsT=I): tanh saturates to -1 and
                # exp(-10) ~ 4.5e-5, well under the accuracy gate, so the
                # mask can ride inside the tanh argument.
                t = sb.tile([128, N], F16, tag=f"t{pr}", name="t")
                for (n0, n1) in ((0, 512), (512, N)):
                    psc = ps_big.tile([128, n1 - n0], F32, tag="psc",
                                      name="psc")
                    for j in range(2):
                        nc.tensor.matmul(
                            psc[64 * j:64 * j + 64, :],
                            lhsT=fqT_t[pr][:, 64 * j:64 * j + 64],
                            rhs=embTs[pr][:, j, n0:n1],
                            start=True, stop=False, skip_group_check=True)
                    nc.tensor.matmul(
                        psc[:], lhsT=ident[:], rhs=distg[pr][:, n0:n1],
                        start=False, stop=False, skip_group_check=True)
                    nc.tensor.matmul(
                        psc[:], lhsT=ident8[:], rhs=mps[pr][:, n0:n1],
                        start=False, stop=True, skip_group_check=True)
                    nc.scalar.activation(t[:, n0:n1], psc[:], AF.Tanh)

                e = sb.tile([128, N], F16, tag=f"e{pr}", name="e")
                ssum = sb.tile([128, 1], F32, tag=f"ssum{pr}", name="ssum")
                nc.scalar.activation(e[:], t[:], AF.Exp, scale=TANH_CLIP,
                                     accum_out=ssum[:])
                rec = sb.tile([128, 1], F32, tag=f"rec{pr}", name="rec")
                nc.vector.reciprocal(out=rec[:], in_=ssum[:])
                nc.vector.tensor_scalar(out=pout[:, pr, :], in0=e[:],
                                        scalar1=rec[:, 0:1], scalar2=None,
                                        op0=OP.mult)
                if pr == 1:
                    nc.sync.dma_start(
                        probs[:, 0:2 * N], pout[:, 0:2, :])
                elif pr == 2:
                    nc.sync.dma_start(
                        probs[:, 2 * N:3 * N], pout[:, 2, :])
                elif pr == 3:
                    nc.scalar.dma_start(
                        probs[:, 3 * N:4 * N], pout[:, 3, :])

            stage_A(0)
            for pr in range(NPAIR):
                stage_B(pr)
                if pr + 1 < NPAIR:
                    stage_A(pr + 1)
                stage_C(pr)

    return nc


_CACHE = {}


def _get_nc():
    if "nc" not in _CACHE:
        _CACHE["nc"] = build_nc()
    return _CACHE["nc"]


def _shard_inputs(inputs):
    f16 = np.float16
    f8 = ml_dtypes.float8_e4m3
    dists = np.asarray(inputs["dists"], dtype=np.float32)
    embeddings = np.asarray(inputs["embeddings"], dtype=np.float32)
    encoded_q1 = np.asarray(inputs["encoded_q1"], dtype=np.float32)
    last_node = np.asarray(inputs["last_node"]).astype(np.int64)
    load = np.asarray(inputs["load"], dtype=np.float32)
    mask = np.asarray(inputs["group_ninf_mask"], dtype=np.float32)
    vis_all = (np.isneginf(mask) | (mask < -1e30))

    wcat = np.zeros((H, 640), f16)
    wcat[:, 0:128] = inputs["Wq_first"].astype(f16)
    wcat[:, 128:256] = inputs["Wq_last"].astype(f16)
    wcat[:, 256:384] = inputs["Wq_graph"].astype(f16)
    wcat[:, 384:512] = inputs["W_visited"].astype(f16)
    wcat[0, 512:640] = inputs["W_load"].astype(f16)
    bldT = (np.asarray(inputs["b_load"], dtype=np.float32) * FQ2) \
        .astype(np.float32).reshape(H, 1)

    in_maps = []
    for c in range(NCORES):
        s = slice(c * NB, (c + 1) * NB)
        emb = embeddings[s]                          # [8,1000,128]
        embT = np.ascontiguousarray(
            emb.transpose(2, 0, 1)).astype(f16).reshape(128, NB * N)
        embp = np.zeros((NB, NPAD, H), f8)
        embp[:, :N] = emb.astype(f8)
        embN = np.ascontiguousarray(
            embp.reshape(NB, NCHUNK, 128, H).transpose(2, 0, 1, 3)
        ).reshape(128, NB * NCHUNK * H)
        embF = np.ascontiguousarray(emb.reshape(NB * N, H).astype(f16))
        distsF = np.zeros((NB * N, NDPAD), f16)
        distsF[:, :N] = (dists[s].reshape(NB * N, N)
                         * np.float32(-INV_SQRT2)).astype(f16)

        vis = vis_all[s]                             # [8,64,1000] bool
        maskP = np.ascontiguousarray(
            (vis.reshape(NPAIR, 128, N).transpose(1, 0, 2))
            .astype(np.float32) * np.float32(MASK_NEG)
        ).astype(f8).reshape(128, NPAIR * N)
        visp = np.zeros((NB, NPAD, P), f8)
        visp[:, :N] = vis.transpose(0, 2, 1).astype(f8)
        maskT = np.concatenate(
            [visp.reshape(NB, NCHUNK, 128, P).transpose(2, 0, 1, 3),
             np.ones((128, NB, NCHUNK, 1), f8)],
            axis=3).reshape(128, NB * NCHUNK * 65)
        maskT = np.ascontiguousarray(maskT)

        eq1T = np.ascontiguousarray(
            encoded_q1[s].astype(f16).transpose(2, 0, 1)
        ).reshape(128, NPAIR * 128)
        idxt = np.ascontiguousarray(
            (last_node[s] + np.arange(NB)[:, None] * N)
            .astype(np.int32).reshape(NPAIR, 128).T)
        loadv = load[s].astype(f16).reshape(1, NPAIR * 128)

        in_maps.append(dict(
            embN=embN, embT=embT, embF=embF, distsF=distsF,
            maskP=maskP, maskT=maskT, eq1T=eq1T, idxt=idxt,
            loadv=loadv, wcat=wcat, bldT=bldT,
        ))
    return in_maps


def _run(inputs, trace=False, **kw):
    nc = _get_nc()
    in_maps = _shard_inputs(inputs)
    res = run_bass_kernel_spmd(nc, in_maps, list(range(NCORES)),
                               trace=trace, **kw)
    out = np.concatenate(
        [r["probs"].astype(np.float32).reshape(128, NPAIR, N)
         .transpose(1, 0, 2).reshape(NB, P, N)
         for r in res.results], axis=0)
    return out, res


def kernel(**inputs) -> np.ndarray:
    out, _ = _run(inputs)
    return out
